# revision 11
# baseline (speedup 1.0000x reference)
"""Trainium2 Bass kernel for nn_CausalSelfAttention_10368051052888.

Head-sharded tensor parallel over 8 NeuronCores (2 heads/core).
Feature-major ("transposed") layout on device: activations live as
[feature, seq] so the PE contraction dim is always the partition dim.

Per core c (heads 2c, 2c+1):
  - qT/kT/vT projections from streamed xT (bf16 matmuls, fp32 PSUM)
  - rmsnorm: per-core sum-of-squares partials + 21KB AllReduce
  - RoPE via DVE stream_shuffle pair swap + host-prepared cos/sin tables,
    applied in place over the staged q/k
  - attention with scores in [k, q] orientation, exp without
    max-subtraction (max|s| ~ 6.5), softmax denominators via DVE
    accumulation + GpSimd partition reduce
  - AllGather of normalized attention outputs (bf16), then column-sharded
    output projection y[:, c*256:(c+1)*256]

Self-contained: hardcodes the problem shapes from the spec.
"""
import numpy as np
import ml_dtypes

import concourse.bass as bass
import concourse.bass_isa as bass_isa
import concourse.mybir as mybir
import concourse.tile as tile
from concourse import bacc
from concourse.bass_utils import run_bass_kernel_spmd

BF = ml_dtypes.bfloat16

N_CORES = 8
S = 2640
D = 2048
H = 16
HD = 128
CACHE = 5280
EPS = 1e-6

HPC = H // N_CORES          # heads per core = 2
MF = HPC * HD               # per-core feature slice = 256
L = CACHE + S               # 7920
KC = D // 128               # 16 contraction chunks
CTILES = (CACHE + 127) // 128   # 42 cache k-tiles (last kt=32)
NTILES = (S + 127) // 128       # 21 new k-tiles / v s-tiles (last 80)
VPAD = CTILES * 128             # 5376 padded cache rows for v
NQ = 512
N_SLICES = [(i * NQ, min(NQ, S - i * NQ)) for i in range((S + NQ - 1) // NQ)]

SWAP_MASK = [(i ^ 1) for i in range(32)]  # pair swap within 32-partition groups

_prog_cache = {}


def build_program():
    dt = mybir.dt
    f32, bf16 = dt.float32, dt.bfloat16
    nc = bacc.Bacc("TRN2", target_bir_lowering=False, debug=False,
                   num_devices=N_CORES)

    # ---------------- I/O ----------------
    xT = nc.dram_tensor("xT", [D, S], bf16, kind="ExternalInput")
    wq = nc.dram_tensor("wq", [D, MF], bf16, kind="ExternalInput")
    wk = nc.dram_tensor("wk", [D, MF], bf16, kind="ExternalInput")
    wv = nc.dram_tensor("wv", [D, MF], bf16, kind="ExternalInput")
    wo = nc.dram_tensor("wo", [D, MF], bf16, kind="ExternalInput")
    cosT = nc.dram_tensor("cosT", [128, S], f32, kind="ExternalInput")
    sinT = nc.dram_tensor("sinT", [128, S], f32, kind="ExternalInput")
    kTc = nc.dram_tensor("kTc", [HPC, 128, CACHE], bf16, kind="ExternalInput")
    vc = nc.dram_tensor("vc", [HPC, 128, VPAD], bf16, kind="ExternalInput")
    masks = nc.dram_tensor("masks", [4, 128, NQ], bf16, kind="ExternalInput")
    yT = nc.dram_tensor("yT", [MF, S], f32, kind="ExternalOutput")

    ssq_in = nc.dram_tensor("ssq_in", [2, S], f32)
    ssq_out = nc.dram_tensor("ssq_out", [2, S], f32, addr_space="Shared")
    ag_in = nc.dram_tensor("ag_in", [MF, S], bf16)
    ag_out = nc.dram_tensor("ag_out", [D, S], bf16, addr_space="Shared")

    RG = [list(range(N_CORES))]
    Exp = mybir.ActivationFunctionType.Exp
    Sqrt = mybir.ActivationFunctionType.Sqrt
    Square = mybir.ActivationFunctionType.Square
    add_op = mybir.AluOpType.add
    mult_op = mybir.AluOpType.mult

    with tile.TileContext(nc) as tc:
        with (
            tc.tile_pool(name="const", bufs=1) as constp,
            tc.tile_pool(name="xs", bufs=4) as xsp,
            tc.tile_pool(name="work", bufs=2) as workp,
            tc.tile_pool(name="ftmp", bufs=4) as ftmp,
            tc.tile_pool(name="attn", bufs=3) as attnp,
            tc.tile_pool(name="psac", bufs=4, space="PSUM") as psac,
            tc.tile_pool(name="pssc", bufs=3, space="PSUM") as pssc,
            tc.tile_pool(name="psq", bufs=1, space="PSUM") as psq,
        ):
            # ------------ persistent SBUF ------------
            w_sb = {}
            for name, tens in (("q", wq), ("k", wk), ("v", wv), ("o", wo)):
                t = constp.tile([128, KC * MF], bf16, tag=f"w{name}", name=f"w{name}")
                nc.sync.dma_start(
                    out=t[:].rearrange("p (kc j) -> p kc j", j=MF),
                    in_=tens[:].rearrange("(kc p) j -> p kc j", p=128),
                )
                w_sb[name] = t
            cos_sb = constp.tile([128, S], f32, tag="cos")
            nc.sync.dma_start(out=cos_sb[:], in_=cosT[:])
            sin_sb = constp.tile([128, S], f32, tag="sin")
            nc.sync.dma_start(out=sin_sb[:], in_=sinT[:])
            mask_sb = constp.tile([128, 4 * NQ], bf16, tag="masks")
            nc.sync.dma_start(
                out=mask_sb[:].rearrange("p (d c) -> p d c", c=NQ),
                in_=masks[:].rearrange("d p c -> p d c"),
            )
            kT_sb = []
            v_sb = []
            for h in range(HPC):
                kt_t = constp.tile([128, L], bf16, tag=f"kT{h}", name=f"kT{h}")
                nc.sync.dma_start(out=kt_t[:, :CACHE], in_=kTc[h])
                kT_sb.append(kt_t)
                v_t = constp.tile([128, VPAD + NTILES * 128], bf16, tag=f"v{h}", name=f"vsb{h}")
                nc.sync.dma_start(out=v_t[:, :VPAD], in_=vc[h])
                v_sb.append(v_t)
            # rq_sb doubles as the q staging buffer (rope runs in place);
            # k stages directly into kT_sb[:, CACHE:].
            rq_sb = [constp.tile([128, S], bf16, tag=f"rq{h}", name=f"rq{h}") for h in range(HPC)]
            onescol = constp.tile([128, 1], bf16, tag="onescol")
            nc.vector.memset(onescol[:], 1.0)
            ssq_q = constp.tile([1, S], f32, tag="ssq_q")
            ssq_k = constp.tile([1, S], f32, tag="ssq_k")
            ssq_t = (ssq_q, ssq_k)
            eps_col = constp.tile([1, 1], f32, tag="eps")
            nc.vector.memset(eps_col[:], EPS)

            def stage_dest(tname, m, qb, nn):
                if tname == "q":
                    return rq_sb[m][:, qb:qb + nn]
                return kT_sb[m][:, CACHE + qb:CACHE + qb + nn]

            # ------------ q/k projection passes ------------
            for ti, tname in enumerate(("q", "k")):
                for (qb, nn) in N_SLICES:
                    pst = [psac.tile([128, NQ], f32, tag="acc", name="pacc_ps")
                           for _ in range(HPC)]
                    for kc in range(KC):
                        xs = xsp.tile([128, NQ], bf16, tag="xs")
                        nc.sync.dma_start(
                            out=xs[:, :nn],
                            in_=xT[kc * 128:(kc + 1) * 128, qb:qb + nn])
                        for m in range(HPC):
                            nc.tensor.matmul(
                                pst[m][:, :nn],
                                w_sb[tname][:, kc * MF + m * 128:
                                            kc * MF + (m + 1) * 128],
                                xs[:, :nn],
                                start=(kc == 0), stop=(kc == KC - 1))
                    # ssq partials: square on ACT, ones-matmul reduce on PE
                    sqp = psq.tile([1, NQ], f32, tag="ssqp")
                    for m in range(HPC):
                        q2 = workp.tile([128, NQ], bf16, tag="btmp")
                        nc.scalar.activation(q2[:, :nn], pst[m][:, :nn], Square)
                        nc.tensor.matmul(sqp[:, :nn], onescol[:], q2[:, :nn],
                                         start=(m == 0), stop=(m == HPC - 1))
                        # stage raw q/k as bf16 for post-AR in-place rope
                        nc.vector.tensor_copy(stage_dest(tname, m, qb, nn),
                                              pst[m][:, :nn])
                    nc.scalar.copy(ssq_t[ti][:, qb:qb + nn], sqp[:, :nn])

            # ------------ ssq AllReduce ------------
            nc.sync.dma_start(out=ssq_in[0:1, :], in_=ssq_q[:])
            nc.sync.dma_start(out=ssq_in[1:2, :], in_=ssq_k[:])
            nc.gpsimd.collective_compute(
                "AllReduce", add_op, replica_groups=RG,
                ins=[ssq_in[:]], outs=[ssq_out[:]])
            nc.sync.dma_start(out=ssq_q[:], in_=ssq_out[0:1, :])
            nc.sync.dma_start(out=ssq_k[:], in_=ssq_out[1:2, :])
            # srow = 1/sqrt(ssq/D + eps), in place
            for t_ in ssq_t:
                nc.scalar.activation(t_[:], t_[:], Sqrt,
                                     scale=1.0 / D, bias=eps_col[:])
                nc.vector.reciprocal(t_[:], t_[:])

            # ------------ v projection pass ------------
            for (qb, nn) in N_SLICES:
                nst = (nn + 127) // 128
                pv = [psac.tile([128, NQ], f32, tag="acc", name="pv_ps")
                      for _ in range(nst)]
                for kc in range(KC):
                    xs = xsp.tile([128, NQ], bf16, tag="xs")
                    nc.sync.dma_start(
                        out=xs[:, :nn],
                        in_=xT[kc * 128:(kc + 1) * 128, qb:qb + nn])
                    for s_ in range(nst):
                        sw = min(128, nn - s_ * 128)
                        nc.tensor.matmul(
                            pv[s_][:sw, :MF],
                            xs[:, s_ * 128:s_ * 128 + sw],
                            w_sb["v"][:, kc * MF:(kc + 1) * MF],
                            start=(kc == 0), stop=(kc == KC - 1))
                for s_ in range(nst):
                    sw = min(128, nn - s_ * 128)
                    st_glob = (qb + s_ * 128) // 128
                    for h in range(HPC):
                        nc.vector.tensor_copy(
                            v_sb[h][:sw, VPAD + st_glob * 128:
                                    VPAD + st_glob * 128 + 128],
                            pv[s_][:sw, h * 128:(h + 1) * 128])

            # ------------ rope, in place over staged q/k ------------
            for ti, tname in enumerate(("q", "k")):
                for m in range(HPC):
                    for (qb, nn) in N_SLICES:
                        st = stage_dest(tname, m, qb, nn)
                        sh = workp.tile([128, NQ], bf16, tag="btmp")
                        nc.vector.stream_shuffle(sh[:, :nn], st, SWAP_MASK)
                        a = ftmp.tile([128, NQ], f32, tag="f32tmp")
                        nc.vector.tensor_tensor(
                            a[:, :nn], st, cos_sb[:, qb:qb + nn], mult_op)
                        b = ftmp.tile([128, NQ], f32, tag="f32tmp")
                        nc.vector.tensor_tensor(
                            b[:, :nn], sh[:, :nn], sin_sb[:, qb:qb + nn],
                            mult_op)
                        nc.vector.tensor_tensor(a[:, :nn], a[:, :nn],
                                                b[:, :nn], add_op)
                        srb = workp.tile([128, NQ], f32, tag="srowb")
                        nc.gpsimd.partition_broadcast(
                            srb[:, :nn], ssq_t[ti][:, qb:qb + nn])
                        nc.vector.tensor_tensor(st, a[:, :nn], srb[:, :nn],
                                                mult_op)

            # ------------ attention ------------
            scale = float(HD) ** -0.5
            for h in range(HPC):
                for (qb, nn) in N_SLICES:
                    # k-tile list: (col0 in kT_sb, kt, vcol0, mask_off or None)
                    tiles = []
                    for ct in range(CTILES):
                        kt = min(128, CACHE - ct * 128)
                        tiles.append((ct * 128, kt, ct * 128, None))
                    for t in range(NTILES):
                        kb = t * 128
                        if kb > qb + nn - 1:
                            continue
                        kt = min(128, S - kb)
                        moff = (kb - qb) if (kb + kt - 1) > qb else None
                        tiles.append((CACHE + kb, kt, VPAD + kb, moff))
                    out_ps = psac.tile([128, NQ], f32, tag="acc", name="out_ps")
                    pacc = attnp.tile([128, NQ], f32, tag="pacc")
                    rq_slice = rq_sb[h][:, qb:qb + nn]
                    nlast = len(tiles) - 1
                    for idx, (c0, kt, vcol, moff) in enumerate(tiles):
                        sc = pssc.tile([128, NQ], f32, tag="scores")
                        nc.tensor.matmul(sc[:kt, :nn],
                                         kT_sb[h][:, c0:c0 + kt],
                                         rq_slice, start=True, stop=True)
                        if moff is not None:
                            mi = moff // 128
                            nc.vector.tensor_tensor(
                                sc[:kt, :nn], sc[:kt, :nn],
                                mask_sb[:kt, mi * NQ:mi * NQ + nn], add_op)
                        pt = attnp.tile([128, NQ], bf16, tag="pT")
                        nc.scalar.activation(pt[:kt, :nn], sc[:kt, :nn], Exp,
                                             scale=scale)
                        if idx == 0:
                            nc.vector.tensor_copy(pacc[:kt, :nn], pt[:kt, :nn])
                        else:
                            nc.vector.tensor_tensor(pacc[:kt, :nn],
                                                    pacc[:kt, :nn],
                                                    pt[:kt, :nn], add_op)
                        nc.tensor.matmul(out_ps[:, :nn],
                                         v_sb[h][:kt, vcol:vcol + 128],
                                         pt[:kt, :nn],
                                         start=(idx == 0), stop=(idx == nlast))
                    recb = attnp.tile([128, NQ], f32, tag="recb")
                    nc.gpsimd.partition_all_reduce(
                        recb[:, :nn], pacc[:, :nn], channels=128,
                        reduce_op=bass_isa.ReduceOp.add)
                    nc.vector.reciprocal(recb[:, :nn], recb[:, :nn])
                    onorm = attnp.tile([128, NQ], bf16, tag="onorm")
                    nc.vector.tensor_tensor(onorm[:, :nn], out_ps[:, :nn],
                                            recb[:, :nn], mult_op)
                    nc.sync.dma_start(
                        out=ag_in[h * 128:(h + 1) * 128, qb:qb + nn],
                        in_=onorm[:, :nn])

            # ------------ AllGather + output projection ------------
            nc.gpsimd.collective_compute(
                "AllGather", mybir.AluOpType.bypass, replica_groups=RG,
                ins=[ag_in[:]], outs=[ag_out[:]])
            for (qb, nn) in N_SLICES:
                py = [psac.tile([128, NQ], f32, tag="acc", name="py_ps")
                      for _ in range(HPC)]
                for kc in range(KC):
                    gt = xsp.tile([128, NQ], bf16, tag="ag")
                    nc.sync.dma_start(
                        out=gt[:, :nn],
                        in_=ag_out[kc * 128:(kc + 1) * 128, qb:qb + nn])
                    for m in range(HPC):
                        nc.tensor.matmul(
                            py[m][:, :nn],
                            w_sb["o"][:, kc * MF + m * 128:
                                      kc * MF + (m + 1) * 128],
                            gt[:, :nn],
                            start=(kc == 0), stop=(kc == KC - 1))
                for m in range(HPC):
                    ys = ftmp.tile([128, NQ], f32, tag="f32tmp")
                    nc.scalar.copy(ys[:, :nn], py[m][:, :nn])
                    nc.sync.dma_start(
                        out=yT[m * 128:(m + 1) * 128, qb:qb + nn],
                        in_=ys[:, :nn])
    nc.compile()
    return nc


def get_program():
    if "nc" not in _prog_cache:
        _prog_cache["nc"] = build_program()
    return _prog_cache["nc"]


def prep_inputs(x, freqs, k_cache, v_cache, Wq, bq, Wk, bk, Wv, bv, Wo, bo,
                gq, gk, current_start):
    """Host-side sharding/layout. Returns per-core in_maps."""
    cs = int(current_start)
    x = np.asarray(x, dtype=np.float32)
    xT = np.ascontiguousarray(x[0].T).astype(BF)           # [D, S]
    freqs = np.asarray(freqs, dtype=np.float32)
    csl = freqs[cs:cs + S, :HD // 2]                       # [S, 64]
    snl = freqs[cs:cs + S, HD // 2:]                       # [S, 64]
    cosT = np.empty((128, S), np.float32)
    sinT = np.empty((128, S), np.float32)
    cosT[0::2] = csl.T
    cosT[1::2] = csl.T
    sinT[0::2] = -snl.T
    sinT[1::2] = snl.T
    # spec guarantees zero biases and unit gains; the device program
    # relies on that (cheap to add back via K=1 bias matmuls if needed)
    for b in (bq, bk, bv, bo):
        assert not np.any(np.asarray(b)), "nonzero bias unsupported"
    for g in (gq, gk):
        assert np.all(np.asarray(g) == 1.0), "non-unit gain unsupported"
    # masks: additive 0 / -1e30, mask_d[r, c] = 0 if c >= r + d
    masks = np.zeros((4, 128, NQ), np.float32)
    r = np.arange(128)[:, None]
    c = np.arange(NQ)[None, :]
    for di, d in enumerate((0, 128, 256, 384)):
        masks[di] = np.where(c >= r + d, 0.0, -1e30)
    masks = masks.astype(BF)

    k_cache = np.asarray(k_cache, np.float32)
    v_cache = np.asarray(v_cache, np.float32)

    in_maps = []
    for core in range(N_CORES):
        h0 = core * HPC
        sl = slice(core * MF, (core + 1) * MF)
        kTc = np.ascontiguousarray(
            np.transpose(k_cache[:, h0:h0 + HPC, :], (1, 2, 0))).astype(BF)
        vpad = np.zeros((HPC, 128, VPAD), BF)
        for h in range(HPC):
            vt = np.zeros((VPAD, HD), np.float32)
            vt[:CACHE] = v_cache[:, h0 + h, :]
            vpad[h] = np.ascontiguousarray(
                vt.reshape(CTILES, 128, HD).transpose(1, 0, 2)
                .reshape(128, VPAD)).astype(BF)
        in_maps.append({
            "xT": xT,
            "wq": np.ascontiguousarray(np.asarray(Wq, np.float32)[sl].T).astype(BF),
            "wk": np.ascontiguousarray(np.asarray(Wk, np.float32)[sl].T).astype(BF),
            "wv": np.ascontiguousarray(np.asarray(Wv, np.float32)[sl].T).astype(BF),
            "wo": np.ascontiguousarray(np.asarray(Wo, np.float32)[sl].T).astype(BF),
            "cosT": cosT,
            "sinT": sinT,
            "kTc": kTc,
            "vc": vpad,
            "masks": masks,
        })
    return in_maps


def assemble_output(results):
    cols = [np.asarray(r["yT"], np.float32).T for r in results]  # [S, MF] each
    return np.ascontiguousarray(np.concatenate(cols, axis=1))[None]


def run(inputs, trace=False):
    nc = get_program()
    in_maps = prep_inputs(**inputs)
    r = run_bass_kernel_spmd(nc, in_maps, core_ids=list(range(N_CORES)),
                             trace=trace)
    return assemble_output(r.results), r


def kernel(**inputs):
    out, _ = run(inputs, trace=False)
    return out
